# revision 11
# baseline (speedup 1.0000x reference)
"""Multi-head attention (b=4, n=2048, embed=768, heads=8) on 8 TRN2 NeuronCores.

Sharding: tensor-parallel over heads — one head per core. Each core computes
its head's Q^T/K^T/V^T projections from the full token stream, runs attention
in a fully "transposed" layout (softmax over the free dim, no attention-matrix
transposes), then a per-batch AllToAll redistributes per-head outputs so each
core owns a 256-token slice of every batch and computes the final projection
(bias folded in via an all-ones lhsT row).

All matmuls run as float32r (4-byte fp32 operands on the fast weight-load
path; ~tf32-class accuracy, ~3x the fp32 matmul rate).
"""

import numpy as np

import concourse.bass as bass
import concourse.tile as tile
from concourse import bacc, mybir
from concourse.bass_utils import run_bass_kernel_spmd
from concourse.masks import make_identity

F32 = mybir.dt.float32
F32R = mybir.dt.float32r

P = 128
EMB = 768
D = 96          # head dim
DP = 97         # head dim + denominator/ones row
NB = 4          # batches
SEQ = 2048      # tokens per batch
QW = 512        # q window width
NWIN = SEQ // QW        # q windows per batch (4)
NCHUNK = SEQ // P       # k chunks per batch (16)
NCORES = 8
NSLOT = SEQ // NCORES   # tokens per (batch, core) output slot (256)
ECHUNK = EMB // P       # embed chunks (6)

_CACHED_NC = None


def _build_nc():
    nc = bacc.Bacc(None, num_devices=NCORES)

    xs = nc.declare_dram_parameter("xs", [NB, 2, P, ECHUNK, SEQ // 2], F32,
                               isOutput=False)
    wc = nc.declare_dram_parameter("wc", [P, ECHUNK, 3, D], F32, isOutput=False)
    bqkv = nc.declare_dram_parameter("bqkv", [D, 3], F32, isOutput=False)
    wp = nc.declare_dram_parameter("wp", [DP, NCORES, EMB], F32, isOutput=False)
    out = nc.declare_dram_parameter("out", [NB, NSLOT, EMB], F32, isOutput=True)

    o_send = nc.dram_tensor("o_send", [NB, NCORES, DP, NSLOT], F32)
    o_recv = nc.dram_tensor("o_recv", [NB, NCORES, DP, NSLOT], F32)

    groups = [list(range(NCORES))]

    with tile.TileContext(nc) as tc:
        with (
            tc.tile_pool(name="singles", bufs=1) as singles,
            tc.tile_pool(name="qkvT", bufs=2) as qkvT_pool,
            tc.tile_pool(name="xwin", bufs=2) as xwin_pool,
            tc.tile_pool(name="vp", bufs=2) as vp_pool,
            tc.tile_pool(name="at", bufs=4) as at_pool,
            tc.tile_pool(name="bc", bufs=2) as bc_pool,
            tc.tile_pool(name="ot", bufs=3) as ot_pool,
            tc.tile_pool(name="po", bufs=2) as po_pool,
            tc.tile_pool(name="lh", bufs=2) as lh_pool,
            tc.tile_pool(name="psA", bufs=2, space="PSUM") as psA,
            tc.tile_pool(name="psS", bufs=2, space="PSUM") as psS,
            tc.tile_pool(name="psO", bufs=1, space="PSUM") as psO,
            tc.tile_pool(name="psP", bufs=1, space="PSUM") as psP,
        ):
            # ---- constants (contiguous per-partition layouts: few big descs) ----
            wc_sb = singles.tile([P, ECHUNK, 3, D], F32R)
            nc.sync.dma_start(out=wc_sb, in_=wc[:, :, :, :].bitcast(F32R))
            bias_sb = singles.tile([D, 3], F32)
            nc.sync.dma_start(out=bias_sb, in_=bqkv[:, :])
            wp_sb = singles.tile([DP, NCORES, EMB], F32R)
            nc.sync.dma_start(out=wp_sb, in_=wp[:, :, :].bitcast(F32R))
            ident = singles.tile([D, D], F32)
            make_identity(nc, ident[:, :])
            ones_sb = singles.tile([P, 1], F32)
            nc.vector.memset(ones_sb[:, :], 1.0)

            def qkv_phase(bb, qkvT):
                qT, kT, vT = qkvT
                for hf in range(2):
                    xt = xwin_pool.tile([P, ECHUNK, SEQ // 2], F32R)
                    nc.sync.dma_start(out=xt, in_=xs[bb, hf].bitcast(F32R))
                    for w2 in range(2):
                        w = hf * 2 + w2
                        ts_ = slice(w2 * QW, (w2 + 1) * QW)
                        for iw, (dstT, bcol) in enumerate(
                            ((qT, 0), (kT, 1), (vT, 2)),
                        ):
                            ps = psA.tile([D, QW], F32, tag="a")
                            for c in range(ECHUNK):
                                nc.tensor.matmul(
                                    ps,
                                    lhsT=wc_sb[:, c, iw, :],
                                    rhs=xt[:, c, ts_],
                                    start=(c == 0), stop=(c == ECHUNK - 1))
                            nc.vector.tensor_scalar_add(
                                out=dstT[:, w * QW:(w + 1) * QW],
                                in0=ps,
                                scalar1=bias_sb[:, bcol:bcol + 1])
                # V' = [ones | V] per 128-token chunk (PE transpose)
                vP = vp_pool.tile([P, NCHUNK, DP], F32R)
                nc.vector.tensor_copy(
                    out=vP[:, :, 0:1],
                    in_=ones_sb[:, 0:1].to_broadcast((P, NCHUNK, 1)))
                for k in range(NCHUNK):
                    pst = psA.tile([P, D], F32, tag="a")
                    nc.tensor.transpose(
                        out=pst, in_=vT[:, k * P:(k + 1) * P].bitcast(F32),
                        identity=ident[:, :])
                    nc.vector.tensor_copy(out=vP[:, k, 1:DP], in_=pst)
                return vP

            def attention_phase(bb, qkvT, vP):
                qT, kT, vT = qkvT
                for w in range(NWIN):
                    qs = slice(w * QW, (w + 1) * QW)
                    ps_o = psO.tile([DP, QW], F32)
                    for kk in range(NCHUNK // 2):
                        ps_s = psS.tile([P, 2, QW], F32)
                        for j in range(2):
                            k = 2 * kk + j
                            nc.tensor.matmul(
                                ps_s[:, j, :],
                                lhsT=kT[:, k * P:(k + 1) * P],
                                rhs=qT[:, qs],
                                start=True, stop=True)
                        aT = at_pool.tile([P, 2, QW], F32R)
                        nc.scalar.activation(
                            out=aT, in_=ps_s,
                            func=mybir.ActivationFunctionType.Exp)
                        for j in range(2):
                            k = 2 * kk + j
                            nc.tensor.matmul(
                                ps_o,
                                lhsT=vP[:, k, :],
                                rhs=aT[:, j, :],
                                start=(k == 0), stop=(k == NCHUNK - 1))
                    # normalize columns by the denominator row (row 0)
                    rec = bc_pool.tile([1, QW], F32, tag="rec")
                    nc.vector.reciprocal(out=rec, in_=ps_o[0:1, :])
                    rbc = bc_pool.tile([DP, QW], F32, tag="rbc")
                    nc.gpsimd.partition_broadcast(rbc[:, :], rec[0:1, :])
                    ot = ot_pool.tile([DP, QW], F32, tag="ot")
                    nc.vector.tensor_mul(out=ot, in0=ps_o, in1=rbc)
                    nc.scalar.dma_start(
                        out=o_send[bb, 2 * w:2 * w + 2, :, :].rearrange(
                            "s p t -> p s t"),
                        in_=ot.rearrange("p (s t) -> p s t", s=2))
                nc.gpsimd.collective_compute(
                    "AllToAll", mybir.AluOpType.bypass,
                    replica_groups=groups,
                    ins=[o_send[bb, :, :, :]],
                    outs=[o_recv[bb, :, :, :]])

            def proj_phase(bb):
                orc = lh_pool.tile([DP, NCORES, NSLOT], F32R)
                nc.sync.dma_start(
                    out=orc,
                    in_=o_recv[bb].bitcast(F32R).rearrange("h p t -> p h t"))
                for t in range(NSLOT // P):
                    po = po_pool.tile([P, 2, 384], F32)
                    for half in range(2):
                        es = slice(half * 384, (half + 1) * 384)
                        pp = psP.tile([P, 384], F32)
                        for h in range(NCORES):
                            nc.tensor.matmul(
                                pp,
                                lhsT=orc[:, h, t * P:(t + 1) * P],
                                rhs=wp_sb[:, h, es],
                                start=(h == 0), stop=(h == NCORES - 1))
                        nc.scalar.mul(out=po[:, half, :], in_=pp, mul=1.0)
                    nc.sync.dma_start(
                        out=out[bb, t * P:(t + 1) * P, :],
                        in_=po.rearrange("p a b -> p (a b)"))

            def new_qkvT():
                qT = qkvT_pool.tile([D, SEQ], F32R, tag="qT")
                kT = qkvT_pool.tile([D, SEQ], F32R, tag="kT")
                vT = qkvT_pool.tile([D, SEQ], F32R, tag="vT")
                return (qT, kT, vT)

            # software pipeline: proj runs one batch behind the A2A
            qkvT = new_qkvT()
            vP = qkv_phase(0, qkvT)
            for bb in range(NB):
                attention_phase(bb, qkvT, vP)
                if bb + 1 < NB:
                    qkvT = new_qkvT()
                    vP = qkv_phase(bb + 1, qkvT)
                proj_phase(bb)

    nc.finalize()
    return nc


def _get_nc():
    global _CACHED_NC
    if _CACHED_NC is None:
        _CACHED_NC = _build_nc()
    return _CACHED_NC


def make_in_maps(x, W_qkv, b_qkv, W_proj, b_proj):
    x = np.asarray(x, dtype=np.float32)
    W_qkv = np.asarray(W_qkv, dtype=np.float32)
    b_qkv = np.asarray(b_qkv, dtype=np.float32)
    W_proj = np.asarray(W_proj, dtype=np.float32)
    b_proj = np.asarray(b_proj, dtype=np.float32)

    scale = 1.0 / np.sqrt(D)
    xT = x.reshape(NB * SEQ, EMB).T                               # [768, 8192]
    # [bb, half, partition, chunk, t] with per-partition-contiguous innermost
    xs = np.ascontiguousarray(
        xT.reshape(ECHUNK, P, NB, 2, SEQ // 2).transpose(2, 3, 1, 0, 4))
    Wr = W_qkv.reshape(EMB, NCORES, D, 3)
    br = b_qkv.reshape(NCORES, D, 3)

    wp_aug = np.zeros((NCORES, DP, EMB), dtype=np.float32)
    wp_aug[:, 1:, :] = W_proj.reshape(NCORES, D, EMB)
    wp_aug[0, 0, :] = b_proj
    # partition-major [97, 8, 768] so the preload is 97 contiguous descriptors
    wp_aug = np.ascontiguousarray(wp_aug.transpose(1, 0, 2))

    in_maps = []
    for h in range(NCORES):
        bias = np.stack(
            [br[h, :, 0] * scale, br[h, :, 1], br[h, :, 2]], axis=1)
        # [128, 6, 3, 96]: per-partition contiguous q/k/v weight slab
        wq_h = (Wr[:, h, :, 0] * scale).reshape(ECHUNK, P, D)
        wk_h = Wr[:, h, :, 1].reshape(ECHUNK, P, D)
        wv_h = Wr[:, h, :, 2].reshape(ECHUNK, P, D)
        wcat = np.ascontiguousarray(
            np.stack([wq_h, wk_h, wv_h], axis=2).transpose(1, 0, 2, 3))
        in_maps.append({
            "xs": xs,
            "wc": wcat,
            "bqkv": np.ascontiguousarray(bias),
            "wp": wp_aug,
        })
    return in_maps


def assemble(results):
    out = np.empty((NB, SEQ, EMB), dtype=np.float32)
    for c in range(NCORES):
        out[:, c * NSLOT:(c + 1) * NSLOT, :] = results[c]["out"]
    return out


def kernel(x, W_qkv, b_qkv, W_proj, b_proj):
    nc = _get_nc()
    in_maps = make_in_maps(x, W_qkv, b_qkv, W_proj, b_proj)
    r = run_bass_kernel_spmd(nc, in_maps, core_ids=list(range(NCORES)))
    return assemble(r.results)


# revision 12
# speedup vs baseline: 1.0166x; 1.0166x over previous
"""Multi-head attention (b=4, n=2048, embed=768, heads=8) on 8 TRN2 NeuronCores.

Sharding: tensor-parallel over heads — one head per core. Each core computes
its head's Q^T/K^T/V^T projections from the full token stream, runs attention
in a fully "transposed" layout (softmax over the free dim, no attention-matrix
transposes), then a per-batch AllToAll redistributes per-head outputs so each
core owns a 256-token slice of every batch and computes the final projection
(bias folded in via an all-ones lhsT row).

All matmuls run as float32r (4-byte fp32 operands on the fast weight-load
path; ~tf32-class accuracy, ~3x the fp32 matmul rate).
"""

import numpy as np

import concourse.bass as bass
import concourse.tile as tile
from concourse import bacc, mybir
from concourse.bass_utils import run_bass_kernel_spmd
from concourse.masks import make_identity

F32 = mybir.dt.float32
F32R = mybir.dt.float32r

P = 128
EMB = 768
D = 96          # head dim
DP = 97         # head dim + denominator/ones row
NB = 4          # batches
SEQ = 2048      # tokens per batch
QW = 512        # q window width
NWIN = SEQ // QW        # q windows per batch (4)
NCHUNK = SEQ // P       # k chunks per batch (16)
NCORES = 8
NSLOT = SEQ // NCORES   # tokens per (batch, core) output slot (256)
ECHUNK = EMB // P       # embed chunks (6)

_CACHED_NC = None


def _build_nc():
    nc = bacc.Bacc(None, num_devices=NCORES)

    xs = nc.declare_dram_parameter("xs", [NB, 2, P, ECHUNK, SEQ // 2], F32,
                               isOutput=False)
    wc = nc.declare_dram_parameter("wc", [P, ECHUNK, 3, D], F32, isOutput=False)
    bqkv = nc.declare_dram_parameter("bqkv", [D, 3], F32, isOutput=False)
    wp = nc.declare_dram_parameter("wp", [DP, NCORES, EMB], F32, isOutput=False)
    out = nc.declare_dram_parameter("out", [NB, NSLOT, EMB], F32, isOutput=True)

    o_send = nc.dram_tensor("o_send", [NB, NCORES, DP, NSLOT], F32)
    o_recv = nc.dram_tensor("o_recv", [NB, NCORES, DP, NSLOT], F32)

    groups = [list(range(NCORES))]

    with tile.TileContext(nc) as tc:
        with (
            tc.tile_pool(name="singles", bufs=1) as singles,
            tc.tile_pool(name="qkvT", bufs=2) as qkvT_pool,
            tc.tile_pool(name="xwin", bufs=2) as xwin_pool,
            tc.tile_pool(name="vp", bufs=2) as vp_pool,
            tc.tile_pool(name="at", bufs=4) as at_pool,
            tc.tile_pool(name="bc", bufs=2) as bc_pool,
            tc.tile_pool(name="ot", bufs=3) as ot_pool,
            tc.tile_pool(name="po", bufs=2) as po_pool,
            tc.tile_pool(name="lh", bufs=2) as lh_pool,
            tc.tile_pool(name="psA", bufs=2, space="PSUM") as psA,
            tc.tile_pool(name="psS", bufs=2, space="PSUM") as psS,
            tc.tile_pool(name="psO", bufs=1, space="PSUM") as psO,
            tc.tile_pool(name="psP", bufs=1, space="PSUM") as psP,
        ):
            # ---- constants (contiguous per-partition layouts: few big descs) ----
            wc_sb = singles.tile([P, ECHUNK, 3, D], F32R)
            nc.sync.dma_start(out=wc_sb, in_=wc[:, :, :, :].bitcast(F32R))
            bias_sb = singles.tile([D, 3], F32)
            nc.sync.dma_start(out=bias_sb, in_=bqkv[:, :])
            wp_sb = singles.tile([DP, NCORES, EMB], F32R)
            nc.scalar.dma_start(out=wp_sb, in_=wp[:, :, :].bitcast(F32R))
            ident = singles.tile([D, D], F32)
            make_identity(nc, ident[:, :])
            ones_sb = singles.tile([P, 1], F32)
            nc.vector.memset(ones_sb[:, :], 1.0)

            def qkv_phase(bb, qkvT):
                qT, kT, vT = qkvT
                for hf in range(2):
                    xt = xwin_pool.tile([P, ECHUNK, SEQ // 2], F32R)
                    nc.gpsimd.dma_start(out=xt, in_=xs[bb, hf].bitcast(F32R))
                    for w2 in range(2):
                        w = hf * 2 + w2
                        ts_ = slice(w2 * QW, (w2 + 1) * QW)
                        for iw, (dstT, bcol) in enumerate(
                            ((qT, 0), (kT, 1), (vT, 2)),
                        ):
                            ps = psA.tile([D, QW], F32, tag="a")
                            for c in range(ECHUNK):
                                nc.tensor.matmul(
                                    ps,
                                    lhsT=wc_sb[:, c, iw, :],
                                    rhs=xt[:, c, ts_],
                                    start=(c == 0), stop=(c == ECHUNK - 1))
                            nc.vector.tensor_scalar_add(
                                out=dstT[:, w * QW:(w + 1) * QW],
                                in0=ps,
                                scalar1=bias_sb[:, bcol:bcol + 1])
                # V' = [ones | V] per 128-token chunk (PE transpose)
                vP = vp_pool.tile([P, NCHUNK, DP], F32R)
                nc.vector.tensor_copy(
                    out=vP[:, :, 0:1],
                    in_=ones_sb[:, 0:1].to_broadcast((P, NCHUNK, 1)))
                for k in range(NCHUNK):
                    pst = psA.tile([P, D], F32, tag="a")
                    nc.tensor.transpose(
                        out=pst, in_=vT[:, k * P:(k + 1) * P].bitcast(F32),
                        identity=ident[:, :])
                    nc.vector.tensor_copy(out=vP[:, k, 1:DP], in_=pst)
                return vP

            def attention_phase(bb, qkvT, vP):
                qT, kT, vT = qkvT
                for w in range(NWIN):
                    qs = slice(w * QW, (w + 1) * QW)
                    ps_o = psO.tile([DP, QW], F32)
                    for kk in range(NCHUNK // 2):
                        ps_s = psS.tile([P, 2, QW], F32)
                        for j in range(2):
                            k = 2 * kk + j
                            nc.tensor.matmul(
                                ps_s[:, j, :],
                                lhsT=kT[:, k * P:(k + 1) * P],
                                rhs=qT[:, qs],
                                start=True, stop=True)
                        aT = at_pool.tile([P, 2, QW], F32R)
                        nc.scalar.activation(
                            out=aT, in_=ps_s,
                            func=mybir.ActivationFunctionType.Exp)
                        for j in range(2):
                            k = 2 * kk + j
                            nc.tensor.matmul(
                                ps_o,
                                lhsT=vP[:, k, :],
                                rhs=aT[:, j, :],
                                start=(k == 0), stop=(k == NCHUNK - 1))
                    # normalize columns by the denominator row (row 0)
                    rec = bc_pool.tile([1, QW], F32, tag="rec")
                    nc.vector.reciprocal(out=rec, in_=ps_o[0:1, :])
                    rbc = bc_pool.tile([DP, QW], F32, tag="rbc")
                    nc.gpsimd.partition_broadcast(rbc[:, :], rec[0:1, :])
                    ot = ot_pool.tile([DP, QW], F32, tag="ot")
                    nc.vector.tensor_mul(out=ot, in0=ps_o, in1=rbc)
                    nc.scalar.dma_start(
                        out=o_send[bb, 2 * w:2 * w + 2, :, :].rearrange(
                            "s p t -> p s t"),
                        in_=ot.rearrange("p (s t) -> p s t", s=2))
                nc.gpsimd.collective_compute(
                    "AllToAll", mybir.AluOpType.bypass,
                    replica_groups=groups,
                    ins=[o_send[bb, :, :, :]],
                    outs=[o_recv[bb, :, :, :]])

            def proj_phase(bb):
                orc = lh_pool.tile([DP, NCORES, NSLOT], F32R)
                nc.gpsimd.dma_start(
                    out=orc,
                    in_=o_recv[bb].bitcast(F32R).rearrange("h p t -> p h t"))
                for t in range(NSLOT // P):
                    po = po_pool.tile([P, 2, 384], F32)
                    for half in range(2):
                        es = slice(half * 384, (half + 1) * 384)
                        pp = psP.tile([P, 384], F32)
                        for h in range(NCORES):
                            nc.tensor.matmul(
                                pp,
                                lhsT=orc[:, h, t * P:(t + 1) * P],
                                rhs=wp_sb[:, h, es],
                                start=(h == 0), stop=(h == NCORES - 1))
                        nc.scalar.mul(out=po[:, half, :], in_=pp, mul=1.0)
                    nc.sync.dma_start(
                        out=out[bb, t * P:(t + 1) * P, :],
                        in_=po.rearrange("p a b -> p (a b)"))

            def new_qkvT():
                qT = qkvT_pool.tile([D, SEQ], F32R, tag="qT")
                kT = qkvT_pool.tile([D, SEQ], F32R, tag="kT")
                vT = qkvT_pool.tile([D, SEQ], F32R, tag="vT")
                return (qT, kT, vT)

            # software pipeline: proj runs one batch behind the A2A
            qkvT = new_qkvT()
            vP = qkv_phase(0, qkvT)
            for bb in range(NB):
                attention_phase(bb, qkvT, vP)
                if bb + 1 < NB:
                    qkvT = new_qkvT()
                    vP = qkv_phase(bb + 1, qkvT)
                proj_phase(bb)

    nc.finalize()
    return nc


def _get_nc():
    global _CACHED_NC
    if _CACHED_NC is None:
        _CACHED_NC = _build_nc()
    return _CACHED_NC


def make_in_maps(x, W_qkv, b_qkv, W_proj, b_proj):
    x = np.asarray(x, dtype=np.float32)
    W_qkv = np.asarray(W_qkv, dtype=np.float32)
    b_qkv = np.asarray(b_qkv, dtype=np.float32)
    W_proj = np.asarray(W_proj, dtype=np.float32)
    b_proj = np.asarray(b_proj, dtype=np.float32)

    scale = 1.0 / np.sqrt(D)
    xT = x.reshape(NB * SEQ, EMB).T                               # [768, 8192]
    # [bb, half, partition, chunk, t] with per-partition-contiguous innermost
    xs = np.ascontiguousarray(
        xT.reshape(ECHUNK, P, NB, 2, SEQ // 2).transpose(2, 3, 1, 0, 4))
    Wr = W_qkv.reshape(EMB, NCORES, D, 3)
    br = b_qkv.reshape(NCORES, D, 3)

    wp_aug = np.zeros((NCORES, DP, EMB), dtype=np.float32)
    wp_aug[:, 1:, :] = W_proj.reshape(NCORES, D, EMB)
    wp_aug[0, 0, :] = b_proj
    # partition-major [97, 8, 768] so the preload is 97 contiguous descriptors
    wp_aug = np.ascontiguousarray(wp_aug.transpose(1, 0, 2))

    in_maps = []
    for h in range(NCORES):
        bias = np.stack(
            [br[h, :, 0] * scale, br[h, :, 1], br[h, :, 2]], axis=1)
        # [128, 6, 3, 96]: per-partition contiguous q/k/v weight slab
        wq_h = (Wr[:, h, :, 0] * scale).reshape(ECHUNK, P, D)
        wk_h = Wr[:, h, :, 1].reshape(ECHUNK, P, D)
        wv_h = Wr[:, h, :, 2].reshape(ECHUNK, P, D)
        wcat = np.ascontiguousarray(
            np.stack([wq_h, wk_h, wv_h], axis=2).transpose(1, 0, 2, 3))
        in_maps.append({
            "xs": xs,
            "wc": wcat,
            "bqkv": np.ascontiguousarray(bias),
            "wp": wp_aug,
        })
    return in_maps


def assemble(results):
    out = np.empty((NB, SEQ, EMB), dtype=np.float32)
    for c in range(NCORES):
        out[:, c * NSLOT:(c + 1) * NSLOT, :] = results[c]["out"]
    return out


def kernel(x, W_qkv, b_qkv, W_proj, b_proj):
    nc = _get_nc()
    in_maps = make_in_maps(x, W_qkv, b_qkv, W_proj, b_proj)
    r = run_bass_kernel_spmd(nc, in_maps, core_ids=list(range(NCORES)))
    return assemble(r.results)


# revision 13
# speedup vs baseline: 1.0969x; 1.0789x over previous
"""Multi-head attention (b=4, n=2048, embed=768, heads=8) on 8 TRN2 NeuronCores.

Sharding: tensor-parallel over heads — one head per core. Each core computes
its head's Q^T/K^T/V^T projections from the full token stream, runs attention
in a fully "transposed" layout (softmax over the free dim, no attention-matrix
transposes), then a per-batch AllToAll redistributes per-head outputs so each
core owns a 256-token slice of every batch and computes the final projection
(bias folded in via an all-ones lhsT row).

All matmuls run as float32r (4-byte fp32 operands on the fast weight-load
path; ~tf32-class accuracy, ~3x the fp32 matmul rate).
"""

import numpy as np

import concourse.bass as bass
import concourse.tile as tile
from concourse import bacc, mybir
from concourse.bass_utils import run_bass_kernel_spmd
from concourse.masks import make_identity

F32 = mybir.dt.float32
F32R = mybir.dt.float32r

P = 128
EMB = 768
D = 96          # head dim
DP = 97         # head dim + denominator/ones row
NB = 4          # batches
SEQ = 2048      # tokens per batch
QW = 512        # q window width
NWIN = SEQ // QW        # q windows per batch (4)
NCHUNK = SEQ // P       # k chunks per batch (16)
NCORES = 8
NSLOT = SEQ // NCORES   # tokens per (batch, core) output slot (256)
ECHUNK = EMB // P       # embed chunks (6)

_CACHED_NC = None


def _build_nc():
    nc = bacc.Bacc(None, num_devices=NCORES)

    xs = nc.declare_dram_parameter("xs", [NB, 2, P, ECHUNK, SEQ // 2], F32,
                               isOutput=False)
    wc = nc.declare_dram_parameter("wc", [P, ECHUNK, 3, D], F32, isOutput=False)
    bqkv = nc.declare_dram_parameter("bqkv", [D, 3], F32, isOutput=False)
    wp = nc.declare_dram_parameter("wp", [DP, NCORES, EMB], F32, isOutput=False)
    out = nc.declare_dram_parameter("out", [NB, NSLOT, EMB], F32, isOutput=True)

    o_send = nc.dram_tensor("o_send", [NB, NCORES, DP, NSLOT], F32)
    o_recv = nc.dram_tensor("o_recv", [NB, NCORES, DP, NSLOT], F32)

    groups = [list(range(NCORES))]

    with tile.TileContext(nc) as tc:
        with (
            tc.tile_pool(name="singles", bufs=1) as singles,
            tc.tile_pool(name="qkvT", bufs=2) as qkvT_pool,
            tc.tile_pool(name="xwin", bufs=2) as xwin_pool,
            tc.tile_pool(name="vp", bufs=2) as vp_pool,
            tc.tile_pool(name="at", bufs=4) as at_pool,
            tc.tile_pool(name="bc", bufs=2) as bc_pool,
            tc.tile_pool(name="ot", bufs=3) as ot_pool,
            tc.tile_pool(name="po", bufs=2) as po_pool,
            tc.tile_pool(name="lh", bufs=2) as lh_pool,
            tc.tile_pool(name="psA", bufs=2, space="PSUM") as psA,
            tc.tile_pool(name="psS", bufs=3, space="PSUM") as psS,
            tc.tile_pool(name="psO", bufs=2, space="PSUM") as psO,
            tc.tile_pool(name="psP", bufs=1, space="PSUM") as psP,
        ):
            # ---- constants (contiguous per-partition layouts: few big descs) ----
            wc_sb = singles.tile([P, ECHUNK, 3, D], F32R)
            nc.sync.dma_start(out=wc_sb, in_=wc[:, :, :, :].bitcast(F32R))
            bias_sb = singles.tile([D, 3], F32)
            nc.sync.dma_start(out=bias_sb, in_=bqkv[:, :])
            wp_sb = singles.tile([DP, NCORES, EMB], F32R)
            ident = singles.tile([D, D], F32)
            make_identity(nc, ident[:, :])
            ones_sb = singles.tile([P, 1], F32)
            nc.vector.memset(ones_sb[:, :], 1.0)

            def qkv_phase(bb, qkvT):
                qT, kT, vT = qkvT
                for hf in range(2):
                    xt = xwin_pool.tile([P, ECHUNK, SEQ // 2], F32R)
                    nc.gpsimd.dma_start(out=xt, in_=xs[bb, hf].bitcast(F32R))
                    for w2 in range(2):
                        w = hf * 2 + w2
                        ts_ = slice(w2 * QW, (w2 + 1) * QW)
                        for iw, (dstT, bcol) in enumerate(
                            ((qT, 0), (kT, 1), (vT, 2)),
                        ):
                            ps = psA.tile([D, QW], F32, tag="a")
                            for c in range(ECHUNK):
                                nc.tensor.matmul(
                                    ps,
                                    lhsT=wc_sb[:, c, iw, :],
                                    rhs=xt[:, c, ts_],
                                    start=(c == 0), stop=(c == ECHUNK - 1))
                            nc.vector.tensor_scalar_add(
                                out=dstT[:, w * QW:(w + 1) * QW],
                                in0=ps,
                                scalar1=bias_sb[:, bcol:bcol + 1])
                # V' = [ones | V] per 128-token chunk (PE transpose)
                vP = vp_pool.tile([P, NCHUNK, DP], F32R)
                nc.vector.tensor_copy(
                    out=vP[:, :, 0:1],
                    in_=ones_sb[:, 0:1].to_broadcast((P, NCHUNK, 1)))
                for k in range(NCHUNK):
                    pst = psA.tile([P, D], F32, tag="a")
                    nc.tensor.transpose(
                        out=pst, in_=vT[:, k * P:(k + 1) * P].bitcast(F32),
                        identity=ident[:, :])
                    nc.vector.tensor_copy(out=vP[:, k, 1:DP], in_=pst)
                return vP

            def attention_phase(bb, qkvT, vP):
                qT, kT, vT = qkvT
                for w in range(NWIN):
                    qs = slice(w * QW, (w + 1) * QW)
                    ps_o = psO.tile([DP, QW], F32)
                    for k in range(NCHUNK):
                        ps_s = psS.tile([P, QW], F32)
                        nc.tensor.matmul(
                            ps_s,
                            lhsT=kT[:, k * P:(k + 1) * P],
                            rhs=qT[:, qs],
                            start=True, stop=True)
                        aT = at_pool.tile([P, QW], F32R)
                        nc.scalar.activation(
                            out=aT, in_=ps_s,
                            func=mybir.ActivationFunctionType.Exp)
                        nc.tensor.matmul(
                            ps_o,
                            lhsT=vP[:, k, :],
                            rhs=aT,
                            start=(k == 0), stop=(k == NCHUNK - 1))
                    # normalize columns by the denominator row (row 0)
                    rec = bc_pool.tile([1, QW], F32, tag="rec")
                    nc.vector.reciprocal(out=rec, in_=ps_o[0:1, :])
                    rbc = bc_pool.tile([DP, QW], F32, tag="rbc")
                    nc.gpsimd.partition_broadcast(rbc[:, :], rec[0:1, :])
                    ot = ot_pool.tile([DP, QW], F32, tag="ot")
                    nc.vector.tensor_mul(out=ot, in0=ps_o, in1=rbc)
                    nc.sync.dma_start(
                        out=o_send[bb, 2 * w, :, :], in_=ot[:, 0:NSLOT])
                    nc.sync.dma_start(
                        out=o_send[bb, 2 * w + 1, :, :], in_=ot[:, NSLOT:QW])
                nc.gpsimd.collective_compute(
                    "AllToAll", mybir.AluOpType.bypass,
                    replica_groups=groups,
                    ins=[o_send[bb, :, :, :]],
                    outs=[o_recv[bb, :, :, :]])

            def proj_phase(bb):
                orc = lh_pool.tile([DP, NCORES, NSLOT], F32R)
                nc.gpsimd.dma_start(
                    out=orc,
                    in_=o_recv[bb].bitcast(F32R).rearrange("h p t -> p h t"))
                for t in range(NSLOT // P):
                    po = po_pool.tile([P, 2, 384], F32)
                    for half in range(2):
                        es = slice(half * 384, (half + 1) * 384)
                        pp = psP.tile([P, 384], F32)
                        for h in range(NCORES):
                            nc.tensor.matmul(
                                pp,
                                lhsT=orc[:, h, t * P:(t + 1) * P],
                                rhs=wp_sb[:, h, es],
                                start=(h == 0), stop=(h == NCORES - 1))
                        nc.scalar.mul(out=po[:, half, :], in_=pp, mul=1.0)
                    nc.sync.dma_start(
                        out=out[bb, t * P:(t + 1) * P, :],
                        in_=po.rearrange("p a b -> p (a b)"))

            def new_qkvT():
                qT = qkvT_pool.tile([D, SEQ], F32R, tag="qT")
                kT = qkvT_pool.tile([D, SEQ], F32R, tag="kT")
                vT = qkvT_pool.tile([D, SEQ], F32R, tag="vT")
                return (qT, kT, vT)

            # software pipeline: proj runs one batch behind the A2A
            qkvT = new_qkvT()
            vP = qkv_phase(0, qkvT)
            nc.gpsimd.dma_start(out=wp_sb, in_=wp[:, :, :].bitcast(F32R))
            for bb in range(NB):
                attention_phase(bb, qkvT, vP)
                if bb + 1 < NB:
                    qkvT = new_qkvT()
                    vP = qkv_phase(bb + 1, qkvT)
                proj_phase(bb)

    nc.finalize()
    return nc


def _get_nc():
    global _CACHED_NC
    if _CACHED_NC is None:
        _CACHED_NC = _build_nc()
    return _CACHED_NC


def make_in_maps(x, W_qkv, b_qkv, W_proj, b_proj):
    x = np.asarray(x, dtype=np.float32)
    W_qkv = np.asarray(W_qkv, dtype=np.float32)
    b_qkv = np.asarray(b_qkv, dtype=np.float32)
    W_proj = np.asarray(W_proj, dtype=np.float32)
    b_proj = np.asarray(b_proj, dtype=np.float32)

    scale = 1.0 / np.sqrt(D)
    xT = x.reshape(NB * SEQ, EMB).T                               # [768, 8192]
    # [bb, half, partition, chunk, t] with per-partition-contiguous innermost
    xs = np.ascontiguousarray(
        xT.reshape(ECHUNK, P, NB, 2, SEQ // 2).transpose(2, 3, 1, 0, 4))
    Wr = W_qkv.reshape(EMB, NCORES, D, 3)
    br = b_qkv.reshape(NCORES, D, 3)

    wp_aug = np.zeros((NCORES, DP, EMB), dtype=np.float32)
    wp_aug[:, 1:, :] = W_proj.reshape(NCORES, D, EMB)
    wp_aug[0, 0, :] = b_proj
    # partition-major [97, 8, 768] so the preload is 97 contiguous descriptors
    wp_aug = np.ascontiguousarray(wp_aug.transpose(1, 0, 2))

    in_maps = []
    for h in range(NCORES):
        bias = np.stack(
            [br[h, :, 0] * scale, br[h, :, 1], br[h, :, 2]], axis=1)
        # [128, 6, 3, 96]: per-partition contiguous q/k/v weight slab
        wq_h = (Wr[:, h, :, 0] * scale).reshape(ECHUNK, P, D)
        wk_h = Wr[:, h, :, 1].reshape(ECHUNK, P, D)
        wv_h = Wr[:, h, :, 2].reshape(ECHUNK, P, D)
        wcat = np.ascontiguousarray(
            np.stack([wq_h, wk_h, wv_h], axis=2).transpose(1, 0, 2, 3))
        in_maps.append({
            "xs": xs,
            "wc": wcat,
            "bqkv": np.ascontiguousarray(bias),
            "wp": wp_aug,
        })
    return in_maps


def assemble(results):
    out = np.empty((NB, SEQ, EMB), dtype=np.float32)
    for c in range(NCORES):
        out[:, c * NSLOT:(c + 1) * NSLOT, :] = results[c]["out"]
    return out


def kernel(x, W_qkv, b_qkv, W_proj, b_proj):
    nc = _get_nc()
    in_maps = make_in_maps(x, W_qkv, b_qkv, W_proj, b_proj)
    r = run_bass_kernel_spmd(nc, in_maps, core_ids=list(range(NCORES)))
    return assemble(r.results)


# revision 15
# speedup vs baseline: 1.1530x; 1.0512x over previous
"""Multi-head attention (b=4, n=2048, embed=768, heads=8) on 8 TRN2 NeuronCores.

Sharding: tensor-parallel over heads — one head per core. Each core computes
its head's Q^T/K^T/V^T projections from the full token stream, runs attention
in a fully "transposed" layout (softmax over the free dim, no attention-matrix
transposes), then a per-batch AllToAll redistributes per-head outputs so each
core owns a 256-token slice of every batch and computes the final projection
(bias folded in via an all-ones lhsT row).

All matmuls run as float32r (4-byte fp32 operands on the fast weight-load
path; ~tf32-class accuracy, ~3x the fp32 matmul rate).
"""

import numpy as np

import concourse.bass as bass
import concourse.tile as tile
from concourse import bacc, mybir
from concourse.bass_utils import run_bass_kernel_spmd
from concourse.masks import make_identity

F32 = mybir.dt.float32
F32R = mybir.dt.float32r

P = 128
EMB = 768
D = 96          # head dim
DP = 97         # head dim + denominator/ones row
NB = 4          # batches
SEQ = 2048      # tokens per batch
QW = 512        # q window width
NWIN = SEQ // QW        # q windows per batch (4)
NCHUNK = SEQ // P       # k chunks per batch (16)
NCORES = 8
NSLOT = SEQ // NCORES   # tokens per (batch, core) output slot (256)
ECHUNK = EMB // P       # embed chunks (6)

_CACHED_NC = None


def _build_nc():
    nc = bacc.Bacc(None, num_devices=NCORES)

    xs = nc.declare_dram_parameter("xs", [NB, 2, P, ECHUNK, SEQ // 2], F32,
                               isOutput=False)
    wc = nc.declare_dram_parameter("wc", [P, ECHUNK, 3, D], F32, isOutput=False)
    bqkv = nc.declare_dram_parameter("bqkv", [D, 3], F32, isOutput=False)
    wp = nc.declare_dram_parameter("wp", [DP, NCORES, EMB], F32, isOutput=False)
    out = nc.declare_dram_parameter("out", [NB, NSLOT, EMB], F32, isOutput=True)

    o_send = nc.dram_tensor("o_send", [NB, NCORES, DP, NSLOT], F32)
    o_recv = nc.dram_tensor("o_recv", [NB, NCORES, DP, NSLOT], F32)

    groups = [list(range(NCORES))]

    with tile.TileContext(nc) as tc:
        with (
            tc.tile_pool(name="singles", bufs=1) as singles,
            tc.tile_pool(name="qkvT", bufs=2) as qkvT_pool,
            tc.tile_pool(name="xwin", bufs=1) as xwin_pool,
            tc.tile_pool(name="vp", bufs=2) as vp_pool,
            tc.tile_pool(name="at", bufs=4) as at_pool,
            tc.tile_pool(name="bc", bufs=2) as bc_pool,
            tc.tile_pool(name="ot", bufs=3) as ot_pool,
            tc.tile_pool(name="po", bufs=2) as po_pool,
            tc.tile_pool(name="lh", bufs=2) as lh_pool,
            tc.tile_pool(name="psA", bufs=2, space="PSUM") as psA,
            tc.tile_pool(name="psS", bufs=3, space="PSUM") as psS,
            tc.tile_pool(name="psO", bufs=2, space="PSUM") as psO,
            tc.tile_pool(name="psP", bufs=1, space="PSUM") as psP,
        ):
            # ---- constants (contiguous per-partition layouts: few big descs) ----
            wc_sb = singles.tile([P, ECHUNK, 3, D], F32R)
            nc.sync.dma_start(out=wc_sb, in_=wc[:, :, :, :].bitcast(F32R))
            bias_sb = singles.tile([D, 3], F32)
            nc.sync.dma_start(out=bias_sb, in_=bqkv[:, :])
            wp_sb = singles.tile([DP, NCORES, EMB], F32R)
            ident = singles.tile([D, D], F32)
            make_identity(nc, ident[:, :])
            ones_sb = singles.tile([P, 1], F32)
            nc.vector.memset(ones_sb[:, :], 1.0)

            def load_x(bb):
                xts = []
                for hf in range(2):
                    xt = xwin_pool.tile([P, ECHUNK, SEQ // 2], F32R,
                                        tag=f"x{hf}")
                    nc.gpsimd.dma_start(out=xt, in_=xs[bb, hf].bitcast(F32R))
                    xts.append(xt)
                return xts

            def qkv_phase(bb, qkvT, xts):
                qT, kT, vT = qkvT
                for hf in range(2):
                    xt = xts[hf]
                    for w2 in range(2):
                        w = hf * 2 + w2
                        ts_ = slice(w2 * QW, (w2 + 1) * QW)
                        for iw, (dstT, bcol) in enumerate(
                            ((qT, 0), (kT, 1), (vT, 2)),
                        ):
                            ps = psA.tile([D, QW], F32, tag="a")
                            for c in range(ECHUNK):
                                nc.tensor.matmul(
                                    ps,
                                    lhsT=wc_sb[:, c, iw, :],
                                    rhs=xt[:, c, ts_],
                                    start=(c == 0), stop=(c == ECHUNK - 1))
                            nc.vector.tensor_scalar_add(
                                out=dstT[:, w * QW:(w + 1) * QW],
                                in0=ps,
                                scalar1=bias_sb[:, bcol:bcol + 1])
                # V' = [ones | V] per 128-token chunk (PE transpose)
                vP = vp_pool.tile([P, NCHUNK, DP], F32R)
                nc.vector.tensor_copy(
                    out=vP[:, :, 0:1],
                    in_=ones_sb[:, 0:1].to_broadcast((P, NCHUNK, 1)))
                for k in range(NCHUNK):
                    pst = psA.tile([P, D], F32, tag="a")
                    nc.tensor.transpose(
                        out=pst, in_=vT[:, k * P:(k + 1) * P].bitcast(F32),
                        identity=ident[:, :])
                    nc.vector.tensor_copy(out=vP[:, k, 1:DP], in_=pst)
                return vP

            def attention_phase(bb, qkvT, vP):
                qT, kT, vT = qkvT
                for w in range(NWIN):
                    qs = slice(w * QW, (w + 1) * QW)
                    ps_o = psO.tile([DP, QW], F32)
                    for k in range(NCHUNK):
                        ps_s = psS.tile([P, QW], F32)
                        nc.tensor.matmul(
                            ps_s,
                            lhsT=kT[:, k * P:(k + 1) * P],
                            rhs=qT[:, qs],
                            start=True, stop=True)
                        aT = at_pool.tile([P, QW], F32R)
                        nc.scalar.activation(
                            out=aT, in_=ps_s,
                            func=mybir.ActivationFunctionType.Exp)
                        nc.tensor.matmul(
                            ps_o,
                            lhsT=vP[:, k, :],
                            rhs=aT,
                            start=(k == 0), stop=(k == NCHUNK - 1))
                    # normalize columns by the denominator row (row 0)
                    rec = bc_pool.tile([1, QW], F32, tag="rec")
                    nc.vector.reciprocal(out=rec, in_=ps_o[0:1, :])
                    rbc = bc_pool.tile([DP, QW], F32, tag="rbc")
                    nc.gpsimd.partition_broadcast(rbc[:, :], rec[0:1, :])
                    ot = ot_pool.tile([DP, QW], F32, tag="ot")
                    nc.vector.tensor_mul(out=ot, in0=ps_o, in1=rbc)
                    nc.sync.dma_start(
                        out=o_send[bb, 2 * w, :, :], in_=ot[:, 0:NSLOT])
                    nc.sync.dma_start(
                        out=o_send[bb, 2 * w + 1, :, :], in_=ot[:, NSLOT:QW])
                nc.gpsimd.collective_compute(
                    "AllToAll", mybir.AluOpType.bypass,
                    replica_groups=groups,
                    ins=[o_send[bb, :, :, :]],
                    outs=[o_recv[bb, :, :, :]])

            def proj_phase(bb):
                orc = lh_pool.tile([DP, NCORES, NSLOT], F32R)
                nc.sync.dma_start(
                    out=orc,
                    in_=o_recv[bb].bitcast(F32R).rearrange("h p t -> p h t"))
                for t in range(NSLOT // P):
                    po = po_pool.tile([P, 2, 384], F32)
                    for half in range(2):
                        es = slice(half * 384, (half + 1) * 384)
                        pp = psP.tile([P, 384], F32)
                        for h in range(NCORES):
                            nc.tensor.matmul(
                                pp,
                                lhsT=orc[:, h, t * P:(t + 1) * P],
                                rhs=wp_sb[:, h, es],
                                start=(h == 0), stop=(h == NCORES - 1))
                        nc.scalar.mul(out=po[:, half, :], in_=pp, mul=1.0)
                    nc.sync.dma_start(
                        out=out[bb, t * P:(t + 1) * P, :],
                        in_=po.rearrange("p a b -> p (a b)"))

            def new_qkvT():
                qT = qkvT_pool.tile([D, SEQ], F32R, tag="qT")
                kT = qkvT_pool.tile([D, SEQ], F32R, tag="kT")
                vT = qkvT_pool.tile([D, SEQ], F32R, tag="vT")
                return (qT, kT, vT)

            # software pipeline: x loads one phase ahead, proj one behind
            xts = load_x(0)
            qkvT = new_qkvT()
            vP = qkv_phase(0, qkvT, xts)
            nc.gpsimd.dma_start(out=wp_sb, in_=wp[:, :, :].bitcast(F32R))
            for bb in range(NB):
                if bb + 1 < NB:
                    xts = load_x(bb + 1)
                attention_phase(bb, qkvT, vP)
                if bb + 1 < NB:
                    qkvT = new_qkvT()
                    vP = qkv_phase(bb + 1, qkvT, xts)
                proj_phase(bb)

    nc.finalize()
    return nc


def _get_nc():
    global _CACHED_NC
    if _CACHED_NC is None:
        _CACHED_NC = _build_nc()
    return _CACHED_NC


def make_in_maps(x, W_qkv, b_qkv, W_proj, b_proj):
    x = np.asarray(x, dtype=np.float32)
    W_qkv = np.asarray(W_qkv, dtype=np.float32)
    b_qkv = np.asarray(b_qkv, dtype=np.float32)
    W_proj = np.asarray(W_proj, dtype=np.float32)
    b_proj = np.asarray(b_proj, dtype=np.float32)

    scale = 1.0 / np.sqrt(D)
    xT = x.reshape(NB * SEQ, EMB).T                               # [768, 8192]
    # [bb, half, partition, chunk, t] with per-partition-contiguous innermost
    xs = np.ascontiguousarray(
        xT.reshape(ECHUNK, P, NB, 2, SEQ // 2).transpose(2, 3, 1, 0, 4))
    Wr = W_qkv.reshape(EMB, NCORES, D, 3)
    br = b_qkv.reshape(NCORES, D, 3)

    wp_aug = np.zeros((NCORES, DP, EMB), dtype=np.float32)
    wp_aug[:, 1:, :] = W_proj.reshape(NCORES, D, EMB)
    wp_aug[0, 0, :] = b_proj
    # partition-major [97, 8, 768] so the preload is 97 contiguous descriptors
    wp_aug = np.ascontiguousarray(wp_aug.transpose(1, 0, 2))

    in_maps = []
    for h in range(NCORES):
        bias = np.stack(
            [br[h, :, 0] * scale, br[h, :, 1], br[h, :, 2]], axis=1)
        # [128, 6, 3, 96]: per-partition contiguous q/k/v weight slab
        wq_h = (Wr[:, h, :, 0] * scale).reshape(ECHUNK, P, D)
        wk_h = Wr[:, h, :, 1].reshape(ECHUNK, P, D)
        wv_h = Wr[:, h, :, 2].reshape(ECHUNK, P, D)
        wcat = np.ascontiguousarray(
            np.stack([wq_h, wk_h, wv_h], axis=2).transpose(1, 0, 2, 3))
        in_maps.append({
            "xs": xs,
            "wc": wcat,
            "bqkv": np.ascontiguousarray(bias),
            "wp": wp_aug,
        })
    return in_maps


def assemble(results):
    out = np.empty((NB, SEQ, EMB), dtype=np.float32)
    for c in range(NCORES):
        out[:, c * NSLOT:(c + 1) * NSLOT, :] = results[c]["out"]
    return out


def kernel(x, W_qkv, b_qkv, W_proj, b_proj):
    nc = _get_nc()
    in_maps = make_in_maps(x, W_qkv, b_qkv, W_proj, b_proj)
    r = run_bass_kernel_spmd(nc, in_maps, core_ids=list(range(NCORES)))
    return assemble(r.results)


# revision 16
# speedup vs baseline: 1.1877x; 1.0301x over previous
"""Multi-head attention (b=4, n=2048, embed=768, heads=8) on 8 TRN2 NeuronCores.

Sharding: tensor-parallel over heads — one head per core. Each core computes
its head's Q^T/K^T/V^T projections from the full token stream, runs attention
in a fully "transposed" layout (softmax over the free dim, no attention-matrix
transposes), then a per-batch AllToAll redistributes per-head outputs so each
core owns a 256-token slice of every batch and computes the final projection
(bias folded in via an all-ones lhsT row).

All matmuls run as float32r (4-byte fp32 operands on the fast weight-load
path; ~tf32-class accuracy, ~3x the fp32 matmul rate).
"""

import numpy as np

import concourse.bass as bass
import concourse.tile as tile
from concourse import bacc, mybir
from concourse.bass_utils import run_bass_kernel_spmd
from concourse.masks import make_identity

F32 = mybir.dt.float32
F32R = mybir.dt.float32r

P = 128
EMB = 768
D = 96          # head dim
DP = 97         # head dim + denominator/ones row
NB = 4          # batches
SEQ = 2048      # tokens per batch
QW = 512        # q window width
NWIN = SEQ // QW        # q windows per batch (4)
NCHUNK = SEQ // P       # k chunks per batch (16)
NCORES = 8
NSLOT = SEQ // NCORES   # tokens per (batch, core) output slot (256)
ECHUNK = EMB // P       # embed chunks (6)

_CACHED_NC = None


def _build_nc():
    nc = bacc.Bacc(None, num_devices=NCORES)

    xs = nc.declare_dram_parameter("xs", [NB, 2, P, ECHUNK, SEQ // 2], F32,
                               isOutput=False)
    wc = nc.declare_dram_parameter("wc", [P, ECHUNK, 3, D], F32, isOutput=False)
    bqkv = nc.declare_dram_parameter("bqkv", [D, 3], F32, isOutput=False)
    wp = nc.declare_dram_parameter("wp", [DP, NCORES, EMB], F32, isOutput=False)
    out = nc.declare_dram_parameter("out", [NB, NSLOT, EMB], F32, isOutput=True)

    o_send = nc.dram_tensor("o_send", [NB, NCORES, DP, NSLOT], F32)
    o_recv = nc.dram_tensor("o_recv", [NB, NCORES, DP, NSLOT], F32)

    groups = [list(range(NCORES))]

    with tile.TileContext(nc) as tc:
        with (
            tc.tile_pool(name="singles", bufs=1) as singles,
            tc.tile_pool(name="qkvT", bufs=2) as qkvT_pool,
            tc.tile_pool(name="xwin", bufs=1) as xwin_pool,
            tc.tile_pool(name="vp", bufs=2) as vp_pool,
            tc.tile_pool(name="at", bufs=4) as at_pool,
            tc.tile_pool(name="bc", bufs=2) as bc_pool,
            tc.tile_pool(name="ot", bufs=3) as ot_pool,
            tc.tile_pool(name="po", bufs=2) as po_pool,
            tc.tile_pool(name="lh", bufs=2) as lh_pool,
            tc.tile_pool(name="psA", bufs=2, space="PSUM") as psA,
            tc.tile_pool(name="psS", bufs=3, space="PSUM") as psS,
            tc.tile_pool(name="psO", bufs=2, space="PSUM") as psO,
            tc.tile_pool(name="psP", bufs=1, space="PSUM") as psP,
        ):
            # ---- constants (contiguous per-partition layouts: few big descs) ----
            wc_sb = singles.tile([P, ECHUNK, 3, D], F32R)
            nc.sync.dma_start(out=wc_sb, in_=wc[:, :, :, :].bitcast(F32R))
            bias_sb = singles.tile([D, 3], F32)
            nc.sync.dma_start(out=bias_sb, in_=bqkv[:, :])
            wp_sb = singles.tile([DP, NCORES, EMB], F32R)
            ident = singles.tile([D, D], F32)
            make_identity(nc, ident[:, :])
            ones_sb = singles.tile([P, 1], F32)
            nc.vector.memset(ones_sb[:, :], 1.0)

            def load_x(bb):
                xts = []
                for hf in range(2):
                    xt = xwin_pool.tile([P, ECHUNK, SEQ // 2], F32R,
                                        tag=f"x{hf}")
                    nc.gpsimd.dma_start(out=xt, in_=xs[bb, hf].bitcast(F32R))
                    xts.append(xt)
                return xts

            def qkv_phase(bb, qkvT, xts):
                qT, kT, vT = qkvT
                for hf in range(2):
                    xt = xts[hf]
                    for w2 in range(2):
                        w = hf * 2 + w2
                        ts_ = slice(w2 * QW, (w2 + 1) * QW)
                        for iw, (dstT, bcol) in enumerate(
                            ((qT, 0), (kT, 1), (vT, 2)),
                        ):
                            ps = psA.tile([D, QW], F32, tag="a")
                            for c in range(ECHUNK):
                                nc.tensor.matmul(
                                    ps,
                                    lhsT=wc_sb[:, c, iw, :],
                                    rhs=xt[:, c, ts_],
                                    start=(c == 0), stop=(c == ECHUNK - 1))
                            nc.vector.tensor_scalar_add(
                                out=dstT[:, w * QW:(w + 1) * QW],
                                in0=ps,
                                scalar1=bias_sb[:, bcol:bcol + 1])
                # V' = [ones | V] per 128-token chunk (PE transpose)
                vP = vp_pool.tile([P, NCHUNK, DP], F32R)
                nc.vector.tensor_copy(
                    out=vP[:, :, 0:1],
                    in_=ones_sb[:, 0:1].to_broadcast((P, NCHUNK, 1)))
                for k in range(NCHUNK):
                    pst = psA.tile([P, D], F32, tag="a")
                    nc.tensor.transpose(
                        out=pst, in_=vT[:, k * P:(k + 1) * P].bitcast(F32),
                        identity=ident[:, :])
                    nc.vector.tensor_copy(out=vP[:, k, 1:DP], in_=pst)
                return vP

            def attention_phase(bb, qkvT, vP):
                qT, kT, vT = qkvT
                for w in range(NWIN):
                    qs = slice(w * QW, (w + 1) * QW)
                    ps_o = psO.tile([DP, QW], F32)
                    for k in range(NCHUNK):
                        ps_s = psS.tile([P, QW], F32)
                        nc.tensor.matmul(
                            ps_s,
                            lhsT=kT[:, k * P:(k + 1) * P],
                            rhs=qT[:, qs],
                            start=True, stop=True)
                        aT = at_pool.tile([P, QW], F32R)
                        nc.scalar.activation(
                            out=aT, in_=ps_s,
                            func=mybir.ActivationFunctionType.Exp)
                        nc.tensor.matmul(
                            ps_o,
                            lhsT=vP[:, k, :],
                            rhs=aT,
                            start=(k == 0), stop=(k == NCHUNK - 1))
                    # normalize columns by the denominator row (row 0)
                    rec = bc_pool.tile([1, QW], F32, tag="rec")
                    nc.vector.reciprocal(out=rec, in_=ps_o[0:1, :])
                    rbc = bc_pool.tile([DP, QW], F32, tag="rbc")
                    nc.gpsimd.partition_broadcast(rbc[:, :], rec[0:1, :])
                    ot = ot_pool.tile([DP, QW], F32, tag="ot")
                    nc.vector.tensor_mul(out=ot, in0=ps_o, in1=rbc)
                    nc.sync.dma_start(
                        out=o_send[bb, 2 * w, :, :], in_=ot[:, 0:NSLOT])
                    nc.sync.dma_start(
                        out=o_send[bb, 2 * w + 1, :, :], in_=ot[:, NSLOT:QW])
                nc.gpsimd.collective_compute(
                    "AllToAll", mybir.AluOpType.bypass,
                    replica_groups=groups,
                    ins=[o_send[bb, :, :, :]],
                    outs=[o_recv[bb, :, :, :]])

            def proj_phase(bb):
                orc = lh_pool.tile([DP, NCORES, NSLOT], F32R)
                nc.sync.dma_start(
                    out=orc,
                    in_=o_recv[bb].bitcast(F32R).rearrange("h p t -> p h t"))
                for t in range(NSLOT // P):
                    po = po_pool.tile([P, 2, 384], F32)
                    for half in range(2):
                        es = slice(half * 384, (half + 1) * 384)
                        pp = psP.tile([P, 384], F32)
                        for h in range(NCORES):
                            nc.tensor.matmul(
                                pp,
                                lhsT=orc[:, h, t * P:(t + 1) * P],
                                rhs=wp_sb[:, h, es],
                                start=(h == 0), stop=(h == NCORES - 1))
                        nc.scalar.mul(out=po[:, half, :], in_=pp, mul=1.0)
                    nc.sync.dma_start(
                        out=out[bb, t * P:(t + 1) * P, :],
                        in_=po.rearrange("p a b -> p (a b)"))

            def new_qkvT():
                qT = qkvT_pool.tile([D, SEQ], F32R, tag="qT")
                kT = qkvT_pool.tile([D, SEQ], F32R, tag="kT")
                vT = qkvT_pool.tile([D, SEQ], F32R, tag="vT")
                return (qT, kT, vT)

            # software pipeline: x loads one phase ahead, proj two behind
            xts = load_x(0)
            qkvT = new_qkvT()
            vP = qkv_phase(0, qkvT, xts)
            for bb in range(NB):
                if bb + 1 < NB:
                    xts = load_x(bb + 1)
                attention_phase(bb, qkvT, vP)
                if bb == 0:
                    nc.gpsimd.dma_start(
                        out=wp_sb, in_=wp[:, :, :].bitcast(F32R))
                if bb + 1 < NB:
                    qkvT = new_qkvT()
                    vP = qkv_phase(bb + 1, qkvT, xts)
                if bb >= 1:
                    proj_phase(bb - 1)
            proj_phase(NB - 1)

    nc.finalize()
    return nc


def _get_nc():
    global _CACHED_NC
    if _CACHED_NC is None:
        _CACHED_NC = _build_nc()
    return _CACHED_NC


def make_in_maps(x, W_qkv, b_qkv, W_proj, b_proj):
    x = np.asarray(x, dtype=np.float32)
    W_qkv = np.asarray(W_qkv, dtype=np.float32)
    b_qkv = np.asarray(b_qkv, dtype=np.float32)
    W_proj = np.asarray(W_proj, dtype=np.float32)
    b_proj = np.asarray(b_proj, dtype=np.float32)

    scale = 1.0 / np.sqrt(D)
    xT = x.reshape(NB * SEQ, EMB).T                               # [768, 8192]
    # [bb, half, partition, chunk, t] with per-partition-contiguous innermost
    xs = np.ascontiguousarray(
        xT.reshape(ECHUNK, P, NB, 2, SEQ // 2).transpose(2, 3, 1, 0, 4))
    Wr = W_qkv.reshape(EMB, NCORES, D, 3)
    br = b_qkv.reshape(NCORES, D, 3)

    wp_aug = np.zeros((NCORES, DP, EMB), dtype=np.float32)
    wp_aug[:, 1:, :] = W_proj.reshape(NCORES, D, EMB)
    wp_aug[0, 0, :] = b_proj
    # partition-major [97, 8, 768] so the preload is 97 contiguous descriptors
    wp_aug = np.ascontiguousarray(wp_aug.transpose(1, 0, 2))

    in_maps = []
    for h in range(NCORES):
        bias = np.stack(
            [br[h, :, 0] * scale, br[h, :, 1], br[h, :, 2]], axis=1)
        # [128, 6, 3, 96]: per-partition contiguous q/k/v weight slab
        wq_h = (Wr[:, h, :, 0] * scale).reshape(ECHUNK, P, D)
        wk_h = Wr[:, h, :, 1].reshape(ECHUNK, P, D)
        wv_h = Wr[:, h, :, 2].reshape(ECHUNK, P, D)
        wcat = np.ascontiguousarray(
            np.stack([wq_h, wk_h, wv_h], axis=2).transpose(1, 0, 2, 3))
        in_maps.append({
            "xs": xs,
            "wc": wcat,
            "bqkv": np.ascontiguousarray(bias),
            "wp": wp_aug,
        })
    return in_maps


def assemble(results):
    out = np.empty((NB, SEQ, EMB), dtype=np.float32)
    for c in range(NCORES):
        out[:, c * NSLOT:(c + 1) * NSLOT, :] = results[c]["out"]
    return out


def kernel(x, W_qkv, b_qkv, W_proj, b_proj):
    nc = _get_nc()
    in_maps = make_in_maps(x, W_qkv, b_qkv, W_proj, b_proj)
    r = run_bass_kernel_spmd(nc, in_maps, core_ids=list(range(NCORES)))
    return assemble(r.results)
